# revision 1
# baseline (speedup 1.0000x reference)
"""Cache-offloaded transformer decode step on 8 TRN2 NeuronCores, v2.

vs baseline: (1) all big loads coalesced to one contiguous DMA per tensor
per layer (host pre-packs KV + weights into device layout), double-buffered
for cross-layer prefetch; (2) the per-layer AllGather/AllReduce collectives
replaced by direct cross-core SBUF writes (7x remote_dma_broadcast + parity
semaphores) — sum semantics, with a per-core selection matrix E placing each
core's attention heads into its XT columns so the sum is an all-gather.
"""

import math
import os
import sys

import numpy as np

for _p in ("/opt/trn_rl_repo",):
    if _p not in sys.path:
        sys.path.append(_p)

import concourse.bass as bass
import concourse.bacc as bacc
import concourse.mybir as mybir
import concourse.tile as tile
from concourse.bass_utils import run_bass_kernel_spmd

F32 = mybir.dt.float32
BF16 = mybir.dt.bfloat16
AF = mybir.ActivationFunctionType
AX = mybir.AxisListType

NCORES = 8
B, S, HID, NH, L, V = 2, 2048, 2048, 16, 8, 32000
D = HID // NH
OFF = (S + 1) // 2                # 1024
R = S - OFF + 1                   # 1025
HPC = NH // NCORES                # 2
PAIRS = B * HPC                   # 4
QKV_PC = 3 * D * HPC              # 768
FFN_PC = 4 * HID // NCORES        # 1024
VPC = V // NCORES                 # 4000
VPCP = 4096
NC16 = HID // 128                 # 16
KCH = 17
KW = 2049                         # keys per pair
VW = KCH * 128                    # 2176 v cols per pair
INVSQ = 1.0 / math.sqrt(D)

# remote_dma engine masks by XOR-distance j (cross-die j&4 -> hi nibbles)
MASKS = {1: 0x0101, 2: 0x0202, 3: 0x0404, 4: 0x1010, 5: 0x2020, 6: 0x4040,
         7: 0x8080}

_cached = None


def _ln(nc, sb, ps, XT, gT, bT, ones, onesr, eps, name):
    """LayerNorm of XT -> (xn_f32, xn_bf16) tiles [128, 32]."""
    sq = sb.tile([128, 32], F32, tag="ln_sq", name=f"sq_{name}")
    nc.vector.tensor_mul(sq[:], XT[:], XT[:])
    st = ps.tile([1, 64], F32, tag="ps_small", name=f"st_{name}")
    nc.vector.memset(st[:], 0.0)
    nc.tensor.matmul(st[0:1, 0:32], ones[:], XT[:], start=False, stop=True)
    nc.tensor.matmul(st[0:1, 32:64], ones[:], sq[:], start=False, stop=True)
    red = sb.tile([1, 4], F32, tag="ln_red", name=f"red_{name}")
    nc.vector.reduce_sum(
        red[:], st[0:1, :].rearrange("p (t b c) -> p (t b) c", t=2, b=2), axis=AX.X
    )
    mr = sb.tile([1, 4], F32, tag="ln_mr", name=f"mr_{name}")
    nc.vector.tensor_scalar_mul(mr[0:1, :], red[0:1, :], 1.0 / HID)
    var = sb.tile([1, 4], F32, tag="ln_var", name=f"var_{name}")
    nc.vector.tensor_mul(var[0:1, 0:2], mr[0:1, 0:2], mr[0:1, 0:2])
    nc.vector.tensor_sub(var[0:1, 0:2], mr[0:1, 2:4], var[0:1, 0:2])
    nc.scalar.activation(var[0:1, 2:4], var[0:1, 0:2], AF.Sqrt,
                         bias=eps[0:1, 0:1])
    nc.vector.reciprocal(mr[0:1, 2:4], var[0:1, 2:4])
    mrb = ps.tile([128, 4], F32, tag="ps_small", name=f"mrb_{name}")
    nc.tensor.matmul(mrb[:], onesr[0:1, :], mr[0:1, :], start=True, stop=True)

    xn = sb.tile([128, 32], F32, tag="xn", name=f"xn_{name}")
    xv = xn[:, :].rearrange("p (b c) -> p b c", b=2)
    XTv = XT[:, :].rearrange("p (b c) -> p b c", b=2)
    m_b = mrb[:, 0:2].unsqueeze(2).broadcast_to([128, 2, 16])
    r_b = mrb[:, 2:4].unsqueeze(2).broadcast_to([128, 2, 16])
    g_b = gT[:, :].unsqueeze(1).broadcast_to([128, 2, 16])
    b_b = bT[:, :].unsqueeze(1).broadcast_to([128, 2, 16])
    nc.vector.tensor_sub(xv, XTv, m_b)
    nc.vector.tensor_mul(xv, xv, r_b)
    nc.vector.tensor_mul(xv, xv, g_b)
    xnb = sb.tile([128, 32], BF16, tag="xnb", name=f"xnb_{name}")
    xnbv = xnb[:, :].rearrange("p (b c) -> p b c", b=2)
    nc.vector.tensor_add(xnbv, xv, b_b)
    return xn, xnb


def _build():
    nc = bacc.Bacc("TRN2", target_bir_lowering=False, debug=False,
                   num_devices=NCORES)

    xpet = nc.dram_tensor("xpet", [128, 32], F32, kind="ExternalInput").ap()
    lng = nc.dram_tensor("lng_t", [128, 16], F32, kind="ExternalInput").ap()
    lnb = nc.dram_tensor("lnb_t", [128, 16], F32, kind="ExternalInput").ap()
    ident_in = nc.dram_tensor("ident", [128, 128], BF16, kind="ExternalInput").ap()
    sel_in = nc.dram_tensor("sel", [4, 32], BF16, kind="ExternalInput").ap()
    qkv_w = nc.dram_tensor("qkv_wt", [L, 128, NC16 * QKV_PC], BF16,
                           kind="ExternalInput").ap()
    f1_w = nc.dram_tensor("ffn1_wt", [L, 128, NC16 * FFN_PC], BF16,
                          kind="ExternalInput").ap()
    f2_w = nc.dram_tensor("ffn2_wt", [L, 128, 8 * HID], BF16,
                          kind="ExternalInput").ap()
    ow = nc.dram_tensor("out_wt", [128, NC16 * VPCP], BF16,
                        kind="ExternalInput").ap()
    ktall_d = nc.dram_tensor("ktall", [L, 128, PAIRS * KW], BF16,
                             kind="ExternalInput").ap()
    vtall_d = nc.dram_tensor("vtall", [L, 128, PAIRS * VW], BF16,
                             kind="ExternalInput").ap()
    out = nc.dram_tensor("out", [128, 64], F32, kind="ExternalOutput").ap()

    rsem = [nc.alloc_semaphore(f"rsem{p}") for p in range(2)]
    lsem = nc.alloc_semaphore("lsem")
    nwait = [0, 0]
    nround = [0]

    with tile.TileContext(nc) as tc:
        with (
            tc.tile_pool(name="sb", bufs=3) as sb,
            tc.tile_pool(name="wq_p", bufs=2) as wq_p,
            tc.tile_pool(name="f1_p", bufs=2) as f1_p,
            tc.tile_pool(name="f2_p", bufs=1) as f2_p,
            tc.tile_pool(name="kt_p", bufs=1) as kt_p,
            tc.tile_pool(name="vt_p", bufs=1) as vt_p,
            tc.tile_pool(name="ps", bufs=3, space="PSUM") as ps,
        ):
            # ---- persistent small tiles ----
            XT = sb.tile([128, 32], F32, bufs=1, name="XT")
            gT = sb.tile([128, 16], F32, bufs=1, name="gT")
            bT = sb.tile([128, 16], F32, bufs=1, name="bT")
            ones = sb.tile([128, 1], F32, bufs=1, name="ones")
            ident = sb.tile([128, 128], BF16, bufs=1, name="ident")
            selm = sb.tile([4, 32], BF16, bufs=1, name="selm")
            nc.scalar.dma_start(XT[:], xpet[:])
            nc.scalar.dma_start(gT[:], lng[:])
            nc.scalar.dma_start(bT[:], lnb[:])
            nc.scalar.dma_start(ident[:], ident_in[:])
            nc.scalar.dma_start(selm[:], sel_in[:])
            nc.vector.memset(ones[:], 1.0)
            onesr = sb.tile([1, 128], F32, bufs=1, name="onesr")
            nc.vector.memset(onesr[:], 1.0)
            eps = sb.tile([1, 1], F32, bufs=1, name="eps")
            nc.vector.memset(eps[:], 1e-5)
            knew = [sb.tile([128, 4], BF16, bufs=1, name=f"knew{l}")
                    for l in range(L - 2)]
            vnew = [sb.tile([4, 128], BF16, bufs=1, name=f"vnew{l}")
                    for l in range(L - 2)]
            # collective buffers: parity double-buffered
            stg = [sb.tile([128, 32], F32, bufs=1, name=f"stg{p}")
                   for p in range(2)]
            agb = [sb.tile([128, 8, 32], F32, bufs=1, name=f"agb{p}")
                   for p in range(2)]

            def allred(src_ps, name):
                """Sum src_ps [128,32] (PSUM) across all 8 cores -> sbuf tile."""
                r = nround[0]
                nround[0] += 1
                p = r & 1
                if r >= 2:
                    with tc.tile_critical():
                        nc.vector.wait_ge(lsem, 112 * (r - 1))
                        nc.vector.tensor_copy(stg[p][:], src_ps)
                else:
                    nc.vector.tensor_copy(stg[p][:], src_ps)
                nc.vector.tensor_copy(agb[p][:, 0, :], stg[p][:])
                for j in range(1, 8):
                    rd = [None] * 8
                    rd[j] = (0, j)
                    nc.gpsimd.remote_dma_broadcast(
                        agb[p][:, j, :], stg[p][:],
                        remote_sem=rsem[p], local_sem=lsem, rdests=rd)
                nc.gpsimd.trigger_dma(count=None)
                nwait[p] += 14
                with tc.tile_critical():
                    nc.vector.wait_ge(rsem[p], nwait[p])
                    res = sb.tile([128, 32], F32, tag="ar_res",
                                  name=f"res_{name}")
                    nc.vector.reduce_sum(
                        res[:], agb[p][:, :, :].rearrange("p s f -> p f s"),
                        axis=AX.X)
                return res

            for l in range(L):
                # ---------- LN1 + QKV ----------
                xn, xnb = _ln(nc, sb, ps, XT, gT, bT, ones, onesr, eps, f"l{l}a")
                wt = wq_p.tile([128, NC16 * QKV_PC], BF16, tag="wq",
                               name=f"wq{l}")
                nc.sync.dma_start(wt[:], qkv_w[l, :, :])
                psq = ps.tile([128, 12], F32, tag="ps_big", name=f"psq{l}")
                nc.vector.memset(psq[:], 0.0)
                for ch in range(NC16):
                    rhs = xnb[:, ch::16]
                    for j in range(6):
                        nc.tensor.matmul(
                            psq[:, 2 * j:2 * j + 2],
                            wt[:, QKV_PC * ch + 128 * j:
                               QKV_PC * ch + 128 * (j + 1)],
                            rhs, start=False, stop=(ch == NC16 - 1),
                        )
                # extract qT, k_new, v_newT; psq col = 6*jh + 2*t + b
                qT = sb.tile([128, 4], BF16, tag="qT", name=f"qT{l}")
                for b in range(2):
                    nc.vector.tensor_copy(qT[:, 2 * b:2 * b + 2],
                                          psq[:, b:b + 7:6])
                if l < L - 2:
                    vnT = sb.tile([128, 4], BF16, tag="vnT", name=f"vnT{l}")
                    for b in range(2):
                        nc.vector.tensor_copy(knew[l][:, 2 * b:2 * b + 2],
                                              psq[:, 2 + b:2 + b + 7:6])
                        nc.vector.tensor_copy(vnT[:, 2 * b:2 * b + 2],
                                              psq[:, 4 + b:4 + b + 7:6])
                    ptr = ps.tile([4, 128], BF16, tag="ps_small",
                                  name=f"ptr{l}")
                    nc.tensor.transpose(ptr[:], vnT[:], ident[:])
                    nc.vector.tensor_copy(vnew[l][:], ptr[:])

                # ---------- attention ----------
                ktall = kt_p.tile([128, PAIRS * KW], BF16, tag="kt",
                                  name=f"kt{l}")
                vtall = vt_p.tile([128, PAIRS * VW], BF16, tag="vt",
                                  name=f"vt{l}")
                nc.scalar.dma_start(ktall[:], ktall_d[l, :, :])
                nc.scalar.dma_start(vtall[:], vtall_d[l, :, :])
                # inject new KV rows computed on-device
                for pi in range(PAIRS):
                    if l == 0:
                        nc.vector.tensor_copy(
                            ktall[:, pi * KW + 2048:pi * KW + 2049],
                            knew[0][:, pi:pi + 1])
                        nc.gpsimd.dma_start(
                            vtall[0:1, pi * VW + 2048:pi * VW + 2176],
                            vnew[0][pi:pi + 1, :])
                    elif l >= 2:
                        nc.vector.tensor_copy(
                            ktall[:, pi * KW + 1023:pi * KW + 1024],
                            knew[l - 2][:, pi:pi + 1])
                        nc.gpsimd.dma_start(
                            vtall[127:128, pi * VW + 896:pi * VW + 1024],
                            vnew[l - 2][pi:pi + 1, :])
                pss = ps.tile([128, PAIRS * KCH], F32, tag="ps_big",
                              name=f"pss{l}")
                nc.vector.memset(pss[:], 0.0)
                for pi in range(PAIRS):
                    for c in range(16):
                        nc.tensor.matmul(
                            pss[:, KCH * pi + c:KCH * pi + c + 1],
                            ktall[:, pi * KW + 128 * c:pi * KW + 128 * (c + 1)],
                            qT[:, pi:pi + 1], start=False, stop=True,
                        )
                    nc.tensor.matmul(
                        pss[0:1, KCH * pi + 16:KCH * pi + 17],
                        ktall[:, pi * KW + 2048:pi * KW + 2049],
                        qT[:, pi:pi + 1], start=False, stop=True,
                    )
                prob = sb.tile([128, PAIRS * KCH], F32, tag="prob",
                               name=f"prob{l}")
                nc.scalar.activation(prob[:], pss[:], AF.Exp, scale=INVSQ)
                ssum = sb.tile([1, 4], F32, tag="ssum", name=f"ssum{l}")
                pssum = ps.tile([1, PAIRS * KCH], F32, tag="ps_small",
                                name=f"pssum{l}")
                nc.tensor.matmul(pssum[:], ones[:], prob[:], start=True,
                                 stop=True)
                sumsb = sb.tile([1, PAIRS * KCH], F32, tag="sumsb",
                                name=f"sumsb{l}")
                nc.vector.tensor_copy(sumsb[:], pssum[:])
                nc.vector.reduce_sum(
                    ssum[:],
                    sumsb[0:1, :].rearrange("p (q c) -> p q c", q=PAIRS),
                    axis=AX.X,
                )
                nc.vector.tensor_scalar_add(ssum[:], ssum[:], -127.0)
                inv = sb.tile([1, 4], F32, tag="inv", name=f"inv{l}")
                nc.vector.reciprocal(inv[:], ssum[:])
                invb = ps.tile([128, 4], F32, tag="ps_small",
                               name=f"invb{l}")
                nc.tensor.matmul(invb[:], onesr[0:1, :], inv[0:1, :],
                                 start=True, stop=True)
                prob_b = sb.tile([128, PAIRS * KCH], BF16, tag="prob_b",
                                 name=f"prob_b{l}")
                nc.vector.tensor_mul(
                    prob_b[:, :].rearrange("p (q c) -> p q c", q=PAIRS),
                    prob[:, :].rearrange("p (q c) -> p q c", q=PAIRS),
                    invb[:, :].unsqueeze(2).broadcast_to([128, PAIRS, KCH]),
                )
                pso = ps.tile([128, 4], F32, tag="ps_big", name=f"pso{l}")
                nc.vector.memset(pso[:], 0.0)
                for pi in range(PAIRS):
                    for c in range(16):
                        nc.tensor.matmul(
                            pso[:, pi:pi + 1],
                            vtall[:, pi * VW + 128 * c:pi * VW + 128 * (c + 1)],
                            prob_b[:, KCH * pi + c:KCH * pi + c + 1],
                            start=False, stop=False,
                        )
                    nc.tensor.matmul(
                        pso[:, pi:pi + 1],
                        vtall[0:1, pi * VW + 2048:pi * VW + 2176],
                        prob_b[0:1, KCH * pi + 16:KCH * pi + 17],
                        start=False, stop=True,
                    )
                o_sb = sb.tile([128, 4], BF16, tag="o_sb", name=f"o{l}")
                nc.vector.tensor_copy(o_sb[:], pso[:])

                # place own heads into XT columns: stage = o @ E
                potr = ps.tile([4, 128], BF16, tag="ps_small", name=f"potr{l}")
                nc.tensor.transpose(potr[:], o_sb[:], ident[:])
                oT = sb.tile([4, 128], BF16, tag="oT", name=f"oT{l}")
                nc.vector.tensor_copy(oT[:], potr[:])
                pstg = ps.tile([128, 32], F32, tag="ps_big", name=f"pstg{l}")
                nc.vector.memset(pstg[:], 0.0)
                nc.tensor.matmul(pstg[:], oT[:], selm[:], start=False,
                                 stop=True)
                delta = allred(pstg[:, :], f"attn{l}")
                nc.vector.tensor_add(XT[:], XT[:], delta[:])

                # ---------- LN2 + FFN ----------
                xn2, xnb2 = _ln(nc, sb, ps, XT, gT, bT, ones, onesr, eps, f"l{l}b")
                wt1 = f1_p.tile([128, NC16 * FFN_PC], BF16, tag="f1",
                                name=f"f1{l}")
                nc.sync.dma_start(wt1[:], f1_w[l, :, :])
                psh = ps.tile([128, 16], F32, tag="ps_big", name=f"psh{l}")
                nc.vector.memset(psh[:], 0.0)
                for ch in range(NC16):
                    rhs = xnb2[:, ch::16]
                    for j in range(8):
                        nc.tensor.matmul(
                            psh[:, 2 * j:2 * j + 2],
                            wt1[:, FFN_PC * ch + 128 * j:
                                FFN_PC * ch + 128 * (j + 1)],
                            rhs, start=False, stop=(ch == NC16 - 1),
                        )
                hT = sb.tile([128, 16], BF16, tag="hT", name=f"hT{l}")
                nc.scalar.activation(hT[:], psh[:], AF.Gelu)
                wt2 = f2_p.tile([128, 8 * HID], BF16, tag="f2", name=f"f2{l}")
                nc.sync.dma_start(wt2[:], f2_w[l, :, :])
                psf = ps.tile([128, 32], F32, tag="ps_big", name=f"psf{l}")
                nc.vector.memset(psf[:], 0.0)
                for ck in range(8):
                    rhs = hT[:, 2 * ck:2 * ck + 2]
                    for m in range(16):
                        nc.tensor.matmul(
                            psf[:, 2 * m:2 * m + 2],
                            wt2[:, HID * ck + 128 * m:HID * ck + 128 * (m + 1)],
                            rhs, start=False, stop=(ck == 7),
                        )
                ard = allred(psf[:, :], f"ffn{l}")
                nc.vector.tensor_add(
                    XT[:, :].rearrange("p (b c) -> p b c", b=2),
                    XT[:, :].rearrange("p (b c) -> p b c", b=2),
                    ard[:, :].rearrange("p (m b) -> p b m", b=2),
                )

            # ---------- final LN + vocab head ----------
            xn3, xnb3 = _ln(nc, sb, ps, XT, gT, bT, ones, onesr, eps, "fin")
            psl = ps.tile([128, 64], F32, tag="ps_big", name="psl")
            nc.vector.memset(psl[:], 0.0)
            for cc in range(4):
                pool = f1_p if cc % 2 == 0 else f2_p
                tag = "f1" if cc % 2 == 0 else "f2"
                wto = pool.tile([128, 4 * VPCP], BF16, tag=tag,
                                name=f"ow{cc}")
                nc.sync.dma_start(wto[:], ow[:, 4 * VPCP * cc:4 * VPCP * (cc + 1)])
                for ci in range(4):
                    c = 4 * cc + ci
                    rhs = xnb3[:, c::16]
                    for m in range(32):
                        nc.tensor.matmul(
                            psl[:, 2 * m:2 * m + 2],
                            wto[:, VPCP * ci + 128 * m:VPCP * ci + 128 * (m + 1)],
                            rhs, start=False, stop=(c == NC16 - 1),
                        )
            logT = sb.tile([128, 64], F32, bufs=1, name="logT")
            nc.vector.tensor_copy(logT[:], psl[:])
            E = sb.tile([128, 64], F32, bufs=1, name="E")
            nc.scalar.activation(E[:], logT[:], AF.Exp)
            nc.sync.dma_start(out[:], E[:])

    nc.compile()
    return nc


def _get_nc():
    global _cached
    if _cached is None:
        _cached = _build()
    return _cached


def _pos_encoding(pos):
    half = np.arange(HID // 2, dtype=np.float32)
    div = np.exp((-math.log(10000.0) * (2.0 * half) / HID).astype(np.float32))
    ang = np.float32(pos) * div
    pe = np.zeros((HID,), dtype=np.float32)
    pe[0::2] = np.sin(ang)
    pe[1::2] = np.cos(ang)
    return pe


def kernel(x, qkv_w, ffn1_w, ffn2_w, out_w, ln_g, ln_b,
           k_heap, v_heap, k_off, v_off, current_pos):
    import ml_dtypes
    bf16 = ml_dtypes.bfloat16

    x = np.asarray(x, dtype=np.float32)
    qkv_w = np.asarray(qkv_w, dtype=np.float32)
    ffn1_w = np.asarray(ffn1_w, dtype=np.float32)
    ffn2_w = np.asarray(ffn2_w, dtype=np.float32)
    out_w = np.asarray(out_w, dtype=np.float32)
    ln_g = np.asarray(ln_g, dtype=np.float32)
    ln_b = np.asarray(ln_b, dtype=np.float32)
    k_heap = np.asarray(k_heap, dtype=np.float32)
    v_heap = np.asarray(v_heap, dtype=np.float32)
    k_off = np.asarray(k_off, dtype=np.float32)
    v_off = np.asarray(v_off, dtype=np.float32)
    pos = int(np.asarray(current_pos))

    xpe = x.reshape(B, HID) + _pos_encoding(pos)[None, :]
    xpet = np.ascontiguousarray(
        xpe.reshape(B, NC16, 128).transpose(2, 0, 1).reshape(128, B * NC16))
    lng_t = np.ascontiguousarray(ln_g.reshape(NC16, 128).T)
    lnb_t = np.ascontiguousarray(ln_b.reshape(NC16, 128).T)
    ident = np.eye(128, dtype=bf16)

    in_maps = []
    for c in range(NCORES):
        # weights: [L, 128, chunks*out] with wt[l, p, ch*O + o] = W[l, o_row, ch*128+p]
        qs = qkv_w[:, QKV_PC * c:QKV_PC * (c + 1), :]        # [L, 768, 2048]
        qs = qs.reshape(L, QKV_PC, NC16, 128).transpose(0, 3, 2, 1)
        qs = np.ascontiguousarray(qs.reshape(L, 128, NC16 * QKV_PC).astype(bf16))
        f1 = ffn1_w[:, FFN_PC * c:FFN_PC * (c + 1), :]       # [L, 1024, 2048]
        f1 = f1.reshape(L, FFN_PC, NC16, 128).transpose(0, 3, 2, 1)
        f1 = np.ascontiguousarray(f1.reshape(L, 128, NC16 * FFN_PC).astype(bf16))
        f2 = ffn2_w[:, :, FFN_PC * c:FFN_PC * (c + 1)]       # [L, 2048, 1024]
        f2 = f2.transpose(0, 2, 1).reshape(L, 8, 128, HID).transpose(0, 2, 1, 3)
        f2 = np.ascontiguousarray(f2.reshape(L, 128, 8 * HID).astype(bf16))
        owt = np.zeros((128, NC16 * VPCP), dtype=bf16)
        ow_c = out_w[VPC * c:VPC * (c + 1), :]               # [4000, 2048]
        ow_r = np.zeros((VPCP, HID), dtype=np.float32)
        ow_r[:VPC] = ow_c
        # owt[p, ch*VPCP + o] = ow_r[o, ch*128+p]
        owt[:] = ow_r.reshape(VPCP, NC16, 128).transpose(2, 1, 0).reshape(
            128, NC16 * VPCP).astype(bf16)

        h0, h1 = HPC * c, HPC * (c + 1)
        kh = k_heap[:, h0:h1].reshape(PAIRS, -1, 128)        # [4, P, 128]
        vh = v_heap[:, h0:h1].reshape(PAIRS, -1, 128)
        ko = k_off[:, :, h0:h1].reshape(L - 1, PAIRS, OFF, 128)
        vo = v_off[:, :, h0:h1].reshape(L - 1, PAIRS, OFF, 128)
        ktall = np.zeros((L, 128, PAIRS * KW), dtype=bf16)
        vtall = np.zeros((L, 128, PAIRS * VW), dtype=bf16)
        for l in range(L):
            if l == 0:
                kc = kh[:, 0:KW]                             # [4, 2049, 128]
                vc = vh[:, 0:KW]
            else:
                rs = (l - 1) * R
                kc = np.concatenate([kh[:, rs:rs + R], ko[l - 1]], axis=1)
                vc = np.concatenate([vh[:, rs:rs + R], vo[l - 1]], axis=1)
            ktall[l] = kc.transpose(2, 0, 1).reshape(128, PAIRS * KW).astype(bf16)
            vp = np.zeros((PAIRS, VW, 128), dtype=np.float32)
            vp[:, :KW] = vc
            # vt[p, pi*VW + chd] with chd = ch*128+d, row = ch*128+p
            vtall[l] = vp.reshape(PAIRS, KCH, 128, 128).transpose(
                2, 0, 1, 3).reshape(128, PAIRS * VW).astype(bf16)

        # selection matrix: E[pi=2b+j, b*16 + 2c + j] = 1
        sel = np.zeros((4, 32), dtype=bf16)
        for b in range(2):
            for j in range(2):
                sel[2 * b + j, b * 16 + 2 * c + j] = 1
        in_maps.append({
            "xpet": xpet, "lng_t": lng_t, "lnb_t": lnb_t, "ident": ident,
            "sel": sel, "qkv_wt": qs, "ffn1_wt": f1, "ffn2_wt": f2,
            "out_wt": owt, "ktall": np.ascontiguousarray(ktall),
            "vtall": np.ascontiguousarray(vtall),
        })

    nc = _get_nc()
    try:
        res = run_bass_kernel_spmd(nc, in_maps, core_ids=list(range(NCORES)))
    except ModuleNotFoundError:
        os.environ["BASS_NEVER_TRACE"] = "1"
        res = run_bass_kernel_spmd(nc, in_maps, core_ids=list(range(NCORES)))
    global LAST_RESULT
    LAST_RESULT = res

    expv = np.zeros((B, V), dtype=np.float32)
    for c in range(NCORES):
        o = res.results[c]["out"].reshape(128, 32, 2)
        for b in range(B):
            expv[b, VPC * c:VPC * (c + 1)] = \
                o[:, :, b].T.reshape(VPCP)[:VPC]
    probs = expv / expv.sum(axis=1, keepdims=True)
    return probs.reshape(B, 1, V).astype(np.float32)

